# revision 19
# baseline (speedup 1.0000x reference)
"""Causal self-attention with RoPE, tensor-parallel over heads on 8 TRN2 cores.

Reference computation (B=1, T=4096, C=1024, H=16, hd=64):
    qkv = x @ w_qkv.T ; split q,k,v ; RoPE(q), RoPE(k) (interleaved pairs)
    scores = q k^T / sqrt(hd) ; causal softmax ; out = attn @ v ; out @ w_out.T

Sharding: head-parallel. Core c owns heads {2c, 2c+1} = 128 qkv channels.
Each core:
  - projects q,k,v for its heads from the full x (contraction over C=1024),
    directly in transposed (channel, token) layout,
  - applies RoPE on-chip (partition pair-swap via stream_shuffle + 2 mul + add),
  - computes causal attention in "sT" layout: sT(kk,qq) = k_blk q^T blocks,
    exp on the scalar engine, p^T @ v_ext matmuls where v_ext carries a ones
    column so the softmax denominator accumulates for free,
  - row-parallel out-projection partials: w_out[:, own_ch] @ attn_own,
  - writes its (1024, T) partial; the host sums the 8 partials.

All matmuls run as float32r (fp32 bits, full-rate PE mode).
"""

import numpy as np

import concourse.bass as bass
import concourse.mybir as mybir
import concourse.tile as tile
from concourse import bacc
from concourse.masks import make_identity

F32 = mybir.dt.float32
F32R = mybir.dt.float32r
EXP = mybir.ActivationFunctionType.Exp

N_HEAD = 16
THETA = 10000.0
N_CORES = 8

# even/odd partition pair swap within each 32-partition group
_SWAP_MASK = [i ^ 1 for i in range(32)]


def _r(ap):
    return ap.bitcast(F32R)


def build_nc(T=4096, C=1024, H=N_HEAD, n_cores=N_CORES, CK=512):
    """Build the per-core SPMD Bass program (same program on all cores; all
    per-core variation is carried by the input tensors)."""
    P = 128
    hd = C // H                    # 64
    hpc = H // n_cores             # heads per core, 2
    CH = hpc * hd                  # per-core qkv channels = 128
    assert CH == P and CK == 512 and T % CK == 0 and C % P == 0
    KS = C // P                    # 8 contraction slices
    NT = T // CK                   # token chunks
    NKT = T // P                   # 128-wide key tiles
    scale = 1.0 / np.sqrt(np.float32(hd))

    nc = bacc.Bacc("TRN2", target_bir_lowering=False, num_devices=n_cores)

    xT = nc.dram_tensor("xT", (C, T), F32R, kind="ExternalInput").ap()
    wqkvT = nc.dram_tensor("wqkvT", (C, 3 * CH), F32R, kind="ExternalInput").ap()
    woutT = nc.dram_tensor("woutT", (CH, C), F32R, kind="ExternalInput").ap()
    cosT = nc.dram_tensor("cosT", (CH, T), F32, kind="ExternalInput").ap()
    sinT = nc.dram_tensor("sinT", (CH, T), F32, kind="ExternalInput").ap()
    partialT = nc.dram_tensor("partialT", (KS, P, T), F32, kind="ExternalOutput").ap()

    with tile.TileContext(nc) as tc:
        with (
            tc.tile_pool(name="const", bufs=1) as const,
            tc.tile_pool(name="persist", bufs=1) as persist,
        ):
            # ---- constants / persistent state ----
            wqkv_sb = const.tile([P, KS, 3 * CH], F32R)
            nc.sync.dma_start(wqkv_sb[:], wqkvT.rearrange("(o p) m -> p o m", p=P))
            wo_sb = [const.tile([hd, C], F32R, tag=f"wo{h}", name=f"wo{h}") for h in range(hpc)]
            for h in range(hpc):
                nc.sync.dma_start(wo_sb[h][:], woutT[h * hd : (h + 1) * hd, :])
            identF = const.tile([P, P], F32)
            make_identity(nc, identF)
            ident = const.tile([P, P], F32R)
            nc.vector.tensor_copy(ident[:], identF[:])
            # causal keep-mask for diagonal 128x128 blocks in (kk, qq) layout:
            # tri[kk, qq] = 1.0 if qq >= kk else 0.0
            tri = const.tile([P, P], F32)
            # ones row (at base partition 64, matching the den row of the att
            # psum tiles) used to broadcast 1/den across partitions via the PE
            onesF = const.tile([hd + 1, hd], F32)
            nc.gpsimd.memset(onesF[hd : hd + 1, :], 1.0)
            ones_bc = const.tile([hd + 1, hd], F32R)
            nc.vector.tensor_copy(ones_bc[hd : hd + 1, :], onesF[hd : hd + 1, :])
            nc.gpsimd.memset(tri[:], 1.0)
            nc.gpsimd.affine_select(
                out=tri[:], in_=tri[:],
                pattern=[[1, P]], base=0, channel_multiplier=-1,
                compare_op=mybir.AluOpType.is_ge, fill=0.0,
            )

            qrot = persist.tile([P, T], F32R)      # [h0 d(64) ; h1 d(64)] x T
            krot = persist.tile([P, T], F32R)
            # v in (token-partition, ki, d+ones) layout per head
            vext = persist.tile([P, hpc, NKT, hd + 1], F32R)
            onesV = const.tile([P, hpc * NKT], F32)
            nc.gpsimd.memset(onesV[:], 1.0)
            nc.vector.tensor_copy(
                vext[:, :, :, hd], onesV[:].rearrange("p (h k) -> p h k", h=hpc)
            )

            # ================= Phase A: qkv projection + RoPE + v transpose
            with (
                tc.tile_pool(name="xcol", bufs=2) as xcol,
                tc.tile_pool(name="trig", bufs=2) as trig,
                tc.tile_pool(name="ptmp", bufs=2) as ptmp,
                tc.tile_pool(name="psA", bufs=3, space="PSUM") as psA,
                tc.tile_pool(name="psT", bufs=2, space="PSUM") as psT,
            ):
                xT_t = xT.rearrange("(o p) n -> p o n", p=P)
                for t in range(NT):
                    tsl = bass.ts(t, CK)
                    xc = xcol.tile([P, KS, CK], F32R, tag="xc", name="xc")
                    nc.sync.dma_start(xc[:], xT_t[:, :, tsl])
                    for w in range(3):  # q, k, v
                        ps = psA.tile([P, CK], F32, tag="psA", name="psA")
                        for ks in range(KS):
                            nc.tensor.matmul(
                                ps[:],
                                wqkv_sb[:, ks, w * CH : (w + 1) * CH],
                                xc[:, ks, :],
                                start=(ks == 0), stop=(ks == KS - 1),
                            )
                        if w < 2:
                            dst = qrot if w == 0 else krot
                            nc.scalar.copy(dst[:, tsl], ps[:])
                        else:
                            vtmp = ptmp.tile([P, CK], F32R, tag="vtmp", name="vtmp")
                            nc.scalar.copy(vtmp[:], ps[:])
                            for tt in range(CK // P):
                                pst = psT.tile([P, P], F32R, tag="pst", name="pst")
                                nc.tensor.transpose(
                                    pst[:], vtmp[:, bass.ts(tt, P)], ident[:]
                                )
                                kt = t * (CK // P) + tt
                                nc.scalar.copy(
                                    vext[:, :, kt, 0:hd],
                                    pst[:].rearrange("p (h d) -> p h d", h=hpc),
                                )
                    # RoPE on this token chunk for q and k
                    cs = trig.tile([P, CK], F32, tag="cos", name="cs")
                    sn = trig.tile([P, CK], F32, tag="sin", name="sn")
                    nc.sync.dma_start(cs[:], cosT[:, tsl])
                    nc.sync.dma_start(sn[:], sinT[:, tsl])
                    for dst in (qrot, krot):
                        sw = ptmp.tile([P, CK], F32, tag="sw", name="sw")
                        nc.vector.stream_shuffle(
                            sw[:], dst[:, tsl].bitcast(F32), _SWAP_MASK
                        )
                        nc.vector.tensor_mul(dst[:, tsl], dst[:, tsl], cs[:])
                        nc.vector.tensor_mul(sw[:], sw[:], sn[:])
                        nc.vector.tensor_add(dst[:, tsl], dst[:, tsl], sw[:])

            # ================= Phase B: attention + out-projection
            with (
                tc.tile_pool(name="pT", bufs=2) as pTp,
                tc.tile_pool(name="an", bufs=2) as anp,
                tc.tile_pool(name="sm", bufs=2) as smp,
                tc.tile_pool(name="ob", bufs=3) as obp,
                tc.tile_pool(name="psS", bufs=1, space="PSUM") as psS,
                tc.tile_pool(name="psB", bufs=1, space="PSUM") as psB,
                tc.tile_pool(name="psO", bufs=2, space="PSUM") as psO,
            ):
                for j in range(NT):
                    jsl = bass.ts(j, CK)
                    nki = (j + 1) * (CK // P)
                    att = [psB.tile([hd + 1, CK], F32, tag=f"att{h}", name=f"att{h}") for h in range(hpc)]
                    for kp in range(nki // 2):
                        sT = [psS.tile([P, 2, CK], F32, tag=f"sT{h}", name=f"sT{h}") for h in range(hpc)]
                        pT = [pTp.tile([P, 2, CK], F32R, tag=f"pT{h}", name=f"pT{h}") for h in range(hpc)]
                        for h in range(hpc):
                            hsl = slice(h * hd, (h + 1) * hd)
                            for s in range(2):
                                ki = 2 * kp + s
                                qoff = max(0, P * ki - CK * j)
                                nc.tensor.matmul(
                                    sT[h][:, s, qoff:],
                                    krot[hsl, bass.ts(ki, P)],
                                    qrot[hsl, j * CK + qoff : (j + 1) * CK],
                                    start=True, stop=True,
                                )
                        qoffs = [max(0, P * (2 * kp + s) - CK * j) for s in range(2)]
                        for h in range(hpc):
                            if qoffs == [0, 0]:
                                nc.scalar.activation(
                                    pT[h][:], sT[h][:], EXP, scale=float(scale)
                                )
                            else:
                                for s in range(2):
                                    nc.scalar.activation(
                                        pT[h][:, s, qoffs[s] :],
                                        sT[h][:, s, qoffs[s] :],
                                        EXP,
                                        scale=float(scale),
                                    )
                        for h in range(hpc):
                            for s in range(2):
                                ki = 2 * kp + s
                                qoff = max(0, P * ki - CK * j)
                                if ki >= (CK // P) * j:  # diagonal block: causal mask
                                    nc.vector.tensor_mul(
                                        pT[h][:, s, qoff : qoff + P],
                                        pT[h][:, s, qoff : qoff + P],
                                        tri[:],
                                    )
                                nc.tensor.matmul(
                                    att[h][:, qoff:],
                                    vext[:, h, ki, :],
                                    pT[h][:, s, qoff:],
                                    start=(ki == 0), stop=(ki == nki - 1),
                                )
                    # normalize: attn_h = att_h[0:hd] / att_h[hd]
                    attn = [anp.tile([hd, CK], F32R, tag=f"attn{h}", name=f"attn{h}") for h in range(hpc)]
                    for h in range(hpc):
                        # 1/den on partition 64, then broadcast to 64 partitions
                        # with a contraction-1 PE matmul (ones_bc.T @ rd).
                        rd = smp.tile([hd + 1, CK], F32R, tag=f"rd{h}", name=f"rd{h}")
                        with nc.allow_low_precision(
                            reason="f32r is fp32-width; reciprocal feeds a PE broadcast"
                        ):
                            nc.vector.reciprocal(
                                rd[hd : hd + 1, :], att[h][hd : hd + 1, :]
                            )
                        rbp = psO.tile([hd, CK], F32, tag="pso", name=f"rbp{h}")
                        nc.tensor.matmul(
                            rbp[:], ones_bc[hd : hd + 1, :], rd[hd : hd + 1, :],
                            start=True, stop=True,
                        )
                        rb = smp.tile([hd, CK], F32, tag=f"rb{h}", name=f"rb{h}")
                        nc.vector.tensor_copy(rb[:], rbp[:])
                        nc.vector.tensor_mul(attn[h][:], att[h][0:hd, :], rb[:])
                    # out-projection partial for this query chunk
                    for ot in range(KS):
                        pso = psO.tile([P, CK], F32, tag="pso", name="pso")
                        for h in range(hpc):
                            nc.tensor.matmul(
                                pso[:],
                                wo_sb[h][:, bass.ts(ot, P)],
                                attn[h][:],
                                start=(h == 0), stop=(h == hpc - 1),
                            )
                        osb = obp.tile([P, CK], F32, tag="osb", name="osb")
                        nc.vector.tensor_copy(osb[:], pso[:])
                        nc.sync.dma_start(partialT[ot, :, jsl], osb[:])

    nc.compile()
    return nc


def host_inputs(x, w_qkv, w_out, token_positions, n_cores=N_CORES):
    """Shard/transform the full inputs into per-core input maps."""
    x = np.asarray(x, dtype=np.float32)
    w_qkv = np.asarray(w_qkv, dtype=np.float32)
    w_out = np.asarray(w_out, dtype=np.float32)
    B, T, C = x.shape
    assert B == 1
    H = N_HEAD
    hd = C // H
    hpc = H // n_cores
    CH = hpc * hd

    xT = np.ascontiguousarray(x[0].T)  # (C, T)

    pos = np.asarray(token_positions).astype(np.float32)  # (T,)
    p = np.arange(hd // 2, dtype=np.float32)
    inv_freq = (1.0 / (THETA ** (2.0 * p / hd))).astype(np.float32)
    ang = pos[:, None] * inv_freq[None, :]          # (T, hd/2)
    cos = np.cos(ang).astype(np.float32).T          # (hd/2, T)
    sin = np.sin(ang).astype(np.float32).T
    # expand to the (CH, T) interleaved-pair layout used on chip
    cosT = np.empty((CH, T), np.float32)
    sinT = np.empty((CH, T), np.float32)
    for d in range(CH):
        dd = d % hd
        pp = dd // 2
        cosT[d] = cos[pp]
        sinT[d] = (-sin[pp]) if dd % 2 == 0 else sin[pp]
    cosT = np.ascontiguousarray(cosT)
    sinT = np.ascontiguousarray(sinT)

    in_maps = []
    for c in range(n_cores):
        csl = slice(CH * c, CH * (c + 1))
        wq = w_qkv[CH * c : CH * (c + 1)]
        wk = w_qkv[C + CH * c : C + CH * (c + 1)]
        wv = w_qkv[2 * C + CH * c : 2 * C + CH * (c + 1)]
        wqkvT = np.ascontiguousarray(np.concatenate([wq, wk, wv], axis=0).T)  # (C, 3CH)
        woutT = np.ascontiguousarray(w_out[:, csl].T)  # (CH, C)
        in_maps.append(
            {"xT": xT, "wqkvT": wqkvT, "woutT": woutT, "cosT": cosT, "sinT": sinT}
        )
    return in_maps


def host_combine(results, T, C):
    """Sum the per-core row-parallel out-projection partials."""
    acc = np.zeros((C, T), np.float32)
    for r in results:
        acc += r["partialT"].reshape(C, T)
    return np.ascontiguousarray(acc.T)[None]  # (1, T, C)


_NC_CACHE = {}


def run(x, w_qkv, w_out, token_positions, trace=False, **spmd_kwargs):
    from concourse.bass_utils import run_bass_kernel_spmd

    B, T, C = x.shape
    key = (T, C)
    if key not in _NC_CACHE:
        _NC_CACHE[key] = build_nc(T=T, C=C)
    nc = _NC_CACHE[key]
    in_maps = host_inputs(x, w_qkv, w_out, token_positions)
    res = run_bass_kernel_spmd(
        nc, in_maps, core_ids=list(range(N_CORES)), trace=trace, **spmd_kwargs
    )
    out = host_combine(res.results, T, C)
    return out, res


def run_timed(x, w_qkv, w_out, token_positions, iters=5):
    """Build the sharded PJRT executable once, device_put inputs once, then
    time repeated executions (min over iters). Returns (out, best_ns, times)."""
    import time

    import jax
    import jax.numpy as jnp
    from jax.sharding import Mesh, PartitionSpec
    from jax.experimental.shard_map import shard_map

    from concourse import bass2jax
    from concourse.bass2jax import _bass_exec_p, install_neuronx_cc_hook

    install_neuronx_cc_hook()

    B, T, C = x.shape
    key = (T, C)
    if key not in _NC_CACHE:
        _NC_CACHE[key] = build_nc(T=T, C=C)
    nc = _NC_CACHE[key]
    in_maps = host_inputs(x, w_qkv, w_out, token_positions)
    n_cores = len(in_maps)

    partition_name = nc.partition_id_tensor.name if nc.partition_id_tensor else None
    in_names, out_names, out_avals = [], [], []
    import concourse.mybir as mybir_

    for alloc in nc.m.functions[0].allocations:
        if not isinstance(alloc, mybir_.MemoryLocationSet):
            continue
        name = alloc.memorylocations[0].name
        if alloc.kind == "ExternalInput":
            if name != partition_name:
                in_names.append(name)
        elif alloc.kind == "ExternalOutput":
            out_names.append(name)
            out_avals.append(
                jax.core.ShapedArray(
                    tuple(alloc.tensor_shape), mybir_.dt.np(alloc.dtype)
                )
            )
    n_params = len(in_names)
    all_in_names = list(in_names) + list(out_names)
    if partition_name is not None:
        all_in_names.append(partition_name)

    def _body(*args):
        operands = list(args)
        if partition_name is not None:
            operands.append(bass2jax.partition_id_tensor())
        return tuple(
            _bass_exec_p.bind(
                *operands,
                out_avals=tuple(out_avals),
                in_names=tuple(all_in_names),
                out_names=tuple(out_names),
                lowering_input_output_aliases=(),
                sim_require_finite=True,
                sim_require_nnan=True,
                nc=nc,
            )
        )

    devices = jax.devices()[:n_cores]
    mesh = Mesh(np.asarray(devices), ("core",))
    n_outs = len(out_names)
    in_specs = (PartitionSpec("core"),) * (n_params + n_outs)
    out_specs = (PartitionSpec("core"),) * n_outs
    donate = tuple(range(n_params, n_params + n_outs))

    fn = jax.jit(
        shard_map(
            _body, mesh=mesh,
            in_specs=in_specs, out_specs=out_specs, check_rep=False,
        ),
        donate_argnums=donate,
        keep_unused=True,
    )

    concat_in = [
        np.concatenate([np.asarray(in_maps[c][nm]) for c in range(n_cores)], axis=0)
        for nm in in_names
    ]
    from jax.sharding import NamedSharding

    sh = NamedSharding(mesh, PartitionSpec("core"))
    dev_in = [jax.device_put(a, sh) for a in concat_in]

    def zero_set():
        return [
            jax.device_put(
                np.zeros((n_cores * a.shape[0], *a.shape[1:]), a.dtype), sh
            )
            for a in out_avals
        ]

    zsets = [zero_set() for _ in range(iters + 1)]
    jax.block_until_ready(zsets)

    out_arrs = fn(*dev_in, *zsets[0])
    jax.block_until_ready(out_arrs)

    times = []
    for i in range(iters):
        t0 = time.perf_counter()
        out_arrs2 = fn(*dev_in, *zsets[i + 1])
        jax.block_until_ready(out_arrs2)
        times.append((time.perf_counter() - t0) * 1e9)
    best_ns = int(min(times))

    results = [
        {
            nm: np.asarray(out_arrs[i]).reshape(n_cores, *out_avals[i].shape)[c]
            for i, nm in enumerate(out_names)
        }
        for c in range(n_cores)
    ]
    out = host_combine(results, T, C)
    return out, best_ns, times


def kernel(x, w_qkv, w_out, token_positions):
    out, _ = run(
        np.asarray(x, np.float32),
        np.asarray(w_qkv, np.float32),
        np.asarray(w_out, np.float32),
        np.asarray(token_positions),
    )
    return out
